# revision 17
# baseline (speedup 1.0000x reference)
"""Trainium2 Bass kernel for nn_AttentionBlock (PixelSNAIL-style attention).

Sharding: 8 cores; pair (2s, 2s+1) handles sample s. Within a pair, positions
(hw=4096) are split into 32 blocks of 128; core parity p owns blocks p::2
(interleaved for load balance, packed locally into 2048 columns).

Each core computes K/Q/V + grn for its 2048 positions; Q/V are pair-AllGathered;
attention scores S = Q^T K for all 4096 q-rows x 2048 local columns, with a
flash-style softmax: per-row local max (clamped at 0, matching the reference's
multiplicative-mask semantics), local exp-sums, a tiny stats AllGather and an
analytic combine.  The fully-masked region (exp(0)=1 entries) is folded in via
suffix sums of beta-scaled V column sums.  Output positions are local, so no
large cross-core reduction is needed.

All 8 cores run ONE program; parity differences live in input data only
(dmask: diagonal-block masks; column interleave done on host).
"""
import numpy as np
import ml_dtypes

import concourse.bass as bass
import concourse.bacc as bacc
import concourse.mybir as mybir
import concourse.tile as tile
from concourse.bass_utils import run_bass_kernel_spmd

BF = mybir.dt.float16
F32 = mybir.dt.float32
NPBF = np.float16
AF = mybir.ActivationFunctionType
OP = mybir.AluOpType
AX = mybir.AxisListType

N, NF, KD, VD = 4, 160, 16, 80
HW = 4096
P = 2048              # local positions per core
NB = 16               # local 128-blocks
NT = 32               # global q tiles
CK, CQ = 169, 166
NCH = 4               # 512-wide position chunks
CH = 512
PAIRS = [[0, 1], [2, 3], [4, 5], [6, 7]]


def _ptiles(c):
    return [(o, min(128, c - o)) for o in range(0, c, 128)]


def _cmap(g):
    """global 128-block g -> sbuf column-block index after pair AllGather."""
    return g // 2 if g % 2 == 0 else NB + g // 2


# ----------------------------------------------------------------------------
# host-side weight prep
# ----------------------------------------------------------------------------

def _wn(p):
    v = np.asarray(p['v'], np.float64)
    g = np.asarray(p['g'], np.float64)
    b = np.asarray(p['b'], np.float64)
    w = v * (g / np.linalg.norm(v, axis=1))[:, None]
    return w, b


def _pad_rows_to_A(w, src_c, dst):
    out = np.zeros((2 * CK, w.shape[1]))
    out[dst, :] = w[:src_c]
    out[[CK + i for i in dst], :] = w[src_c:]
    return out


def host_prepare(x, ul, b, params):
    x = np.asarray(x, np.float32)
    ul = np.asarray(ul, np.float32)
    b = np.asarray(b, np.float32)

    W = {}

    def nin_eff(p, fold):
        w, bb = _wn(p)
        if fold:
            bb = bb - w.sum(axis=0)   # device computes elu(+-x)+1
        return w, bb

    w, bb = nin_eff(params['grn_k']['conv_input'], True)
    W['w_ci_k'], W['b_ci_k'] = w, bb
    w, bb = nin_eff(params['grn_v']['conv_input'], True)
    W['w_ci_v'], W['b_ci_v'] = w, bb
    w, bb = nin_eff(params['grn_q']['conv_input'], True)
    W['w_ci_q'], W['b_ci_q'] = _pad_rows_to_A(w, CQ, list(range(3, 169))), bb
    w, bb = nin_eff(params['grn_out']['conv_input'], True)
    wsk, bsk = nin_eff(params['grn_out']['nin_skip'], True)
    W['w_ci_o'] = _pad_rows_to_A(w, NF, list(range(3, 163)))
    W['b_ci_o'] = bb + bsk
    W['w_sk'] = wsk

    for nm, key in (('k', 'grn_k'), ('v', 'grn_v'), ('q', 'grn_q'), ('o', 'grn_out')):
        w, bb = nin_eff(params[key]['conv_out'], True)
        nf2 = bb.shape[0] // 2
        W[f'w_co_{nm}'] = w
        W[f'b_co_{nm}_aa'], W[f'b_co_{nm}_bb'] = bb[:nf2], 0.5 * bb[nf2:]

    for nm in ('k', 'q', 'v'):
        w, bb = nin_eff(params[f'nin_{nm}'], False)
        W[f'w_n{nm}'], W[f'b_n{nm}'] = w, bb

    shared = {}
    for k, v in W.items():
        if k.startswith('b_'):
            shared[k] = np.ascontiguousarray(v.astype(np.float32).reshape(-1, 1))
        else:
            shared[k] = np.ascontiguousarray(v.astype(NPBF))

    r = np.arange(128)
    shared['ident'] = np.eye(128, dtype=NPBF)
    strict_u = (r[None, :] > r[:, None]).astype(np.float32)
    cnt = np.zeros((128, NT), np.float32)
    for t in range(NT):
        cnt[:, t] = 128 * t - (128 if t % 2 == 1 else 0)
    shared['cnt'] = cnt

    xubf = np.concatenate([x.reshape(N, 3, HW), ul.reshape(N, NF, HW),
                           b.reshape(N, 6, HW)], axis=1)      # (N, 169, HW)
    cores = []
    for c in range(8):
        s, p = c // 2, c % 2
        cols = np.concatenate([np.arange(128 * (2 * k + p), 128 * (2 * k + p) + 128)
                               for k in range(NB)])
        m = dict(shared)
        xs = xubf[s][:, cols]
        m['xub'] = np.ascontiguousarray(xs.astype(NPBF))
        m['ulb'] = np.ascontiguousarray(xs[3:169].astype(NPBF))
        m['ulf'] = np.ascontiguousarray(xs[3:163], np.float32)
        if p == 0:
            dm = np.concatenate([strict_u, np.zeros((128, 128), np.float32)], axis=1)
        else:
            dm = np.concatenate([np.ones((128, 128), np.float32), strict_u], axis=1)
        m['dmask'] = np.ascontiguousarray(dm.astype(NPBF))
        cores.append(m)
    return cores


# ----------------------------------------------------------------------------
# device kernel
# ----------------------------------------------------------------------------

WSPEC = {
    'w_ci_k': (2 * CK, CK), 'w_ci_v': (2 * CK, CK),
    'w_ci_q': (2 * CK, CQ), 'w_ci_o': (2 * CK, NF),
    'w_co_k': (2 * CK, 2 * CK), 'w_co_v': (2 * CK, 2 * CK),
    'w_co_q': (2 * CQ, 2 * CQ), 'w_co_o': (2 * NF, 2 * NF),
    'w_nk': (CK, KD), 'w_nq': (CQ, KD), 'w_nv': (CK, VD),
    'w_sk': (2 * VD, NF),
}
BSPEC = {
    'b_ci_k': CK, 'b_ci_v': CK, 'b_ci_q': CQ, 'b_ci_o': NF,
    'b_co_k_aa': CK, 'b_co_k_bb': CK, 'b_co_v_aa': CK, 'b_co_v_bb': CK,
    'b_co_q_aa': CQ, 'b_co_q_bb': CQ, 'b_co_o_aa': NF, 'b_co_o_bb': NF,
    'b_nk': KD, 'b_nq': KD, 'b_nv': VD,
}
WKT = {   # contraction row tiling per weight, matching activation tiles
    'w_ci_k': [(0, 128), (128, 41), (CK, 128), (CK + 128, 41)],
    'w_ci_v': [(0, 128), (128, 41), (CK, 128), (CK + 128, 41)],
    'w_ci_q': [(0, 128), (128, 41), (CK, 128), (CK + 128, 41)],
    'w_ci_o': [(0, 128), (128, 41), (CK, 128), (CK + 128, 41)],
    'w_co_k': [(0, 128), (128, 41), (CK, 128), (CK + 128, 41)],
    'w_co_v': [(0, 128), (128, 41), (CK, 128), (CK + 128, 41)],
    'w_co_q': [(0, 128), (128, 38), (CQ, 128), (CQ + 128, 38)],
    'w_co_o': [(0, 128), (128, 32), (NF, 128), (NF + 128, 32)],
    'w_sk': [(0, VD), (VD, VD)],
    'w_nk': [(0, 128), (128, 41)],
    'w_nq': [(0, 128), (128, 38)],
    'w_nv': [(0, 128), (128, 41)],
}


def build(debug=False):
    nc = bacc.Bacc("TRN2", target_bir_lowering=False, debug=debug,
                   enable_asserts=False, num_devices=8)

    def din(name, shape, dt):
        return nc.dram_tensor(name, shape, dt, kind="ExternalInput")

    d = {}
    d['xub'] = din('xub', [CK, P], BF)
    d['ulb'] = din('ulb', [CQ, P], BF)
    d['ulf'] = din('ulf', [NF, P], F32)
    d['dmask'] = din('dmask', [128, 256], BF)
    d['cnt'] = din('cnt', [128, NT], F32)
    d['ident'] = din('ident', [128, 128], BF)
    d['wd'] = {k: din(k, list(s), BF) for k, s in WSPEC.items()}
    d['bd'] = {k: din(k, [s, 1], F32) for k, s in BSPEC.items()}
    d['out'] = nc.dram_tensor('out', [NF, P], F32, kind="ExternalOutput")
    d['qb'] = nc.dram_tensor('qb', [KD, P], BF)
    d['vb'] = nc.dram_tensor('vb', [VD, P], BF)
    d['stb'] = nc.dram_tensor('stb', [128, 64], F32)
    d['qag'] = nc.dram_tensor('qag', [2 * KD, P], BF)
    d['vag'] = nc.dram_tensor('vag', [2 * VD, P], BF)
    d['stag'] = nc.dram_tensor('stag', [256, 64], F32)

    with tile.TileContext(nc) as tc:
        _body(nc, tc, d)
    nc.compile()
    return nc


def _bias_slice(b_tiles, off, size):
    ti, ro = off // 128, off % 128
    assert ro + size <= b_tiles[ti].shape[0], (off, size)
    return b_tiles[ti][ro:ro + size, 0:1]


def _body(nc, tc, d):
    wd, bd = d['wd'], d['bd']

    with (tc.tile_pool(name="wp", bufs=1) as wp,
          tc.tile_pool(name="sp", bufs=1) as sp):

        # ---- weights / consts / persistent inputs --------------------------
        wsb, bsb = {}, {}
        for k, t in wd.items():
            tiles = []
            for i, (o, s) in enumerate(WKT[k]):
                tl = wp.tile([s, t.shape[1]], BF, tag=f"{k}_{i}")
                nc.sync.dma_start(tl[:], t[o:o + s, :])
                tiles.append(tl)
            wsb[k] = tiles
        for k, t in bd.items():
            tiles = []
            for i, (o, s) in enumerate(_ptiles(t.shape[0])):
                tl = wp.tile([s, 1], F32, tag=f"{k}_{i}")
                nc.sync.dma_start(tl[:], t[o:o + s, :])
                tiles.append(tl)
            bsb[k] = tiles

        dmask_s = wp.tile([128, 256], BF, tag="dmask")
        nc.sync.dma_start(dmask_s[:], d['dmask'][:, :])
        cnt_s = wp.tile([128, NT], F32, tag="cnt")
        nc.sync.dma_start(cnt_s[:], d['cnt'][:, :])
        ident_s = wp.tile([128, 128], BF, tag="ident")
        nc.sync.dma_start(ident_s[:], d['ident'][:, :])

        ulf_t = []
        for i, (o, s) in enumerate(_ptiles(NF)):
            tl = sp.tile([s, P], F32, tag=f"ulf{i}")
            nc.sync.dma_start(tl[:], d['ulf'][o:o + s, :])
            ulf_t.append(tl)

        # ---- helpers -------------------------------------------------------
        def concat_elu(pool, src_tiles, tag, fd=P, bufs_tag=None):
            """src bf16 sbuf tiles -> (e1, e2) with e = elu(+-x)+1."""
            bt = bufs_tag or tag
            e1, e2 = [], []
            for i, s in enumerate(src_tiles):
                sh = [s.shape[0], fd]
                mx = pool.tile(sh, BF, tag=f"{bt}_mx", bufs=1, name="ce_mx")
                a = pool.tile(sh, BF, tag=f"{bt}_a", bufs=1, name="ce_a")
                w = pool.tile(sh, BF, tag=f"{bt}_w", bufs=1, name="ce_w")
                o1 = pool.tile(sh, BF, tag=f"{bt}_e1_{i}", name="ce_e1")
                o2 = pool.tile(sh, BF, tag=f"{bt}_e2_{i}", name="ce_e2")
                nc.vector.tensor_scalar(mx[:], s[:], -1.0, None, OP.mult)
                nc.vector.tensor_tensor(a[:], s[:], mx[:], OP.min)
                nc.scalar.activation(w[:], a[:], AF.Exp)
                nc.vector.scalar_tensor_tensor(o1[:], s[:], 1.0, w[:], OP.add, OP.max)
                nc.vector.scalar_tensor_tensor(o2[:], mx[:], 1.0, w[:], OP.add, OP.max)
                e1.append(o1)
                e2.append(o2)
            return e1, e2

        def nin_mm(pool, w_tiles, act_tiles, m_tiles, ps_tag, fd=P):
            outs = []
            for mo, ms in m_tiles:
                pt = pool.tile([ms, fd], F32, tag=ps_tag)
                for c in range(0, fd, CH):
                    sl = slice(c, min(fd, c + CH))
                    for ki in range(len(w_tiles)):
                        nc.tensor.matmul(
                            pt[:, sl], w_tiles[ki][:, mo:mo + ms],
                            act_tiles[ki][:, sl],
                            start=(ki == 0), stop=(ki == len(w_tiles) - 1))
                outs.append(pt)
            return outs

        # ================= GRN stage (branch-major) =========================
        x1o_t = []
        k_loc = sp.tile([KD, P], BF, tag="k_loc")

        with tc.tile_pool(name="gp", bufs=1) as gp, \
             tc.tile_pool(name="gpp", bufs=1) as gpp, \
             tc.tile_pool(name="pp", bufs=2, space="PSUM") as pp:

            xub_t, ulb_t = [], []
            for i, (o, s) in enumerate(_ptiles(CK)):
                tl = gpp.tile([s, P], BF, tag=f"xub{i}")
                nc.sync.dma_start(tl[:], d['xub'][o:o + s, :])
                xub_t.append(tl)
            for i, (o, s) in enumerate(_ptiles(CQ)):
                tl = gpp.tile([s, P], BF, tag=f"ulb{i}")
                nc.sync.dma_start(tl[:], d['ulb'][o:o + s, :])
                ulb_t.append(tl)

            e1A, e2A = concat_elu(gpp, xub_t, "A")
            A_kt = [e1A[0], e1A[1], e2A[0], e2A[1]]

            def branch(nm, orig_tiles, nf, w_ci, b_ci, w_co, b_co):
                ci = nin_mm(pp, wsb[w_ci], A_kt, _ptiles(nf), "mm")
                xs = []
                for i, ps in enumerate(ci):
                    xsb = gp.tile(list(ps.shape), BF, tag="mx", bufs=2)
                    nc.vector.tensor_scalar(xsb[:], ps[:], bsb[b_ci][i][:, 0:1],
                                            None, OP.add)
                    xs.append(xsb)
                e1, e2 = concat_elu(gp, xs, f"m{nm}", bufs_tag="me")
                ekt = [e1[0], e1[1], e2[0], e2[1]]
                # bb first: sigmoid consumes + releases PSUM before aa tiles
                bb = nin_mm(pp, wsb[w_co], ekt,
                            [(nf, 128), (nf + 128, nf - 128)], "mm")
                sigs = []
                for i in range(2):
                    ms = orig_tiles[i].shape[0]
                    sig = gp.tile([ms, P], F32, tag=f"sig{i}")
                    nc.scalar.activation(sig[:], bb[i][:], AF.Tanh, scale=0.5,
                                         bias=_bias_slice(bsb[b_co + '_bb'], i * 128, ms))
                    nc.vector.tensor_scalar(sig[:], sig[:], 0.5, 0.5, OP.mult, OP.add)
                    sigs.append(sig)
                aa = nin_mm(pp, wsb[w_co], ekt,
                            [(0, 128), (128, nf - 128)], "mm")
                outs = []
                for i in range(2):
                    ms = orig_tiles[i].shape[0]
                    gg = gp.tile([ms, P], BF, tag="gg", bufs=2)
                    xo = gp.tile([ms, P], BF, tag=f"xo{i}")
                    nc.vector.scalar_tensor_tensor(
                        gg[:], aa[i][:], _bias_slice(bsb[b_co + '_aa'], i * 128, ms),
                        sigs[i][:], OP.add, OP.mult)
                    nc.vector.tensor_tensor(xo[:], gg[:], orig_tiles[i][:], OP.add)
                    outs.append(xo)
                return outs

            def proj(w_n, b_n, act_tiles, dout, dst_sb):
                ps = nin_mm(pp, wsb[w_n], act_tiles, [(0, dout)], "mm")[0]
                nc.vector.tensor_scalar(dst_sb[:], ps[:], bsb[b_n][0][:, 0:1],
                                        None, OP.add)

            xq = branch('q', ulb_t, CQ, 'w_ci_q', 'b_ci_q', 'w_co_q', 'b_co_q')
            q_loc = gpp.tile([KD, P], BF, tag="q_loc")
            proj('w_nq', 'b_nq', xq, KD, q_loc)
            nc.sync.dma_start(d['qb'][:, :], q_loc[:])
            nc.gpsimd.collective_compute(
                "AllGather", OP.bypass, replica_groups=PAIRS,
                ins=[d['qb'].ap().opt()], outs=[d['qag'].ap().opt()])

            xv = branch('v', xub_t, CK, 'w_ci_v', 'b_ci_v', 'w_co_v', 'b_co_v')
            v_loc = gpp.tile([VD, P], BF, tag="v_loc")
            proj('w_nv', 'b_nv', xv, VD, v_loc)
            nc.sync.dma_start(d['vb'][:, :], v_loc[:])
            nc.gpsimd.collective_compute(
                "AllGather", OP.bypass, replica_groups=PAIRS,
                ins=[d['vb'].ap().opt()], outs=[d['vag'].ap().opt()])

            xk = branch('k', xub_t, CK, 'w_ci_k', 'b_ci_k', 'w_co_k', 'b_co_k')
            proj('w_nk', 'b_nk', xk, KD, k_loc)

            # conv_input for grn_out (skip accumulated later)
            cio = nin_mm(pp, wsb['w_ci_o'], A_kt, _ptiles(NF), "mm")
            for i, ps in enumerate(cio):
                xsb = sp.tile(list(ps.shape), BF, tag=f"x1o{i}")
                nc.vector.tensor_scalar(xsb[:], ps[:], bsb['b_ci_o'][i][:, 0:1],
                                        None, OP.add)
                x1o_t.append(xsb)

        # ================= attention ========================================
        mc = sp.tile([128, NT], F32, tag="mc")
        negm = sp.tile([128, NT], F32, tag="negm")
        dsum = sp.tile([128, NT], F32, tag="dsum")
        e_tiles = []

        ep_ctx = tc.tile_pool(name="ep", bufs=1)
        ep = ep_ctx.__enter__()
        v_T = [ep.tile([128, VD], BF, tag=f"vT{c}", name=f"vT{c}") for c in range(NT)]

        with tc.tile_pool(name="gc", bufs=1) as gc:
            q_all = gc.tile([KD, HW], BF, tag="q_all")
            nc.sync.dma_start(q_all[:, 0:P], d['qag'][0:KD, :])
            nc.sync.dma_start(q_all[:, P:HW], d['qag'][KD:2 * KD, :])
            v_all = gc.tile([VD, HW], BF, tag="v_all")
            nc.sync.dma_start(v_all[:, 0:P], d['vag'][0:VD, :])
            nc.sync.dma_start(v_all[:, P:HW], d['vag'][VD:2 * VD, :])

            with tc.tile_pool(name="vtpp", bufs=2, space="PSUM") as vt_pp:
                for c in range(NT):
                    ps = vt_pp.tile([128, VD], BF, tag="vtps")
                    nc.tensor.transpose(ps[:], v_all[:, 128 * c:128 * (c + 1)],
                                        ident_s[:VD, :VD])
                    nc.vector.tensor_copy(v_T[c][:], ps[:])

            with tc.tile_pool(name="spp", bufs=2, space="PSUM") as s_pp:
                for t in range(NT):
                    ks = t // 2
                    w = P - 128 * ks
                    sps = s_pp.tile([128, P], F32, tag="s_ps")
                    for c in range(0, w, CH):
                        c1 = min(w, c + CH)
                        nc.tensor.matmul(
                            sps[:, c:c1],
                            q_all[:, 128 * _cmap(t):128 * _cmap(t) + 128],
                            k_loc[:, 128 * ks + c:128 * ks + c1],
                            start=True, stop=True)
                    moff = 128 * (t % 2)
                    nc.vector.tensor_mul(sps[:, 0:128], sps[:, 0:128],
                                         dmask_s[:, moff:moff + 128])
                    nc.vector.tensor_reduce(mc[:, t:t + 1], sps[:, 0:w],
                                            axis=AX.X, op=OP.max)
                    nc.vector.tensor_scalar(negm[:, t:t + 1], mc[:, t:t + 1],
                                            0.0, -1.0, OP.max, OP.mult)
                    et = ep.tile([128, w], BF, tag=f"E{t}")
                    nc.scalar.activation(et[:], sps[:, 0:w], AF.Exp,
                                         bias=negm[:, t:t + 1],
                                         accum_out=dsum[:, t:t + 1])
                    e_tiles.append(et)

        # stats exchange + combine
        nc.vector.tensor_scalar(mc[:], negm[:], -1.0, None, OP.mult)
        nc.sync.dma_start(d['stb'][:, 0:NT], mc[:])
        nc.sync.dma_start(d['stb'][:, NT:2 * NT], dsum[:])
        nc.gpsimd.collective_compute(
            "AllGather", OP.bypass, replica_groups=PAIRS,
            ins=[d['stb'].ap().opt()], outs=[d['stag'].ap().opt()])
        st_ev = sp.tile([128, 64], F32, tag="st_ev")
        st_od = sp.tile([128, 64], F32, tag="st_od")
        nc.sync.dma_start(st_ev[:], d['stag'][0:128, :])
        nc.sync.dma_start(st_od[:], d['stag'][128:256, :])

        Mx = sp.tile([128, NT], F32, tag="Mx")
        tmp = sp.tile([128, NT], F32, tag="tmp")
        tmp2 = sp.tile([128, NT], F32, tag="tmp2")
        D = sp.tile([128, NT], F32, tag="D")
        alpha = sp.tile([128, NT], F32, tag="alpha")
        beta = sp.tile([128, NT], BF, tag="beta")
        rD = sp.tile([128, NT], F32, tag="rD")
        nc.vector.tensor_max(Mx[:], st_ev[:, 0:NT], st_od[:, 0:NT])
        nc.vector.tensor_sub(tmp[:], st_ev[:, 0:NT], Mx[:])
        nc.scalar.activation(tmp[:], tmp[:], AF.Exp)
        nc.vector.tensor_mul(tmp[:], tmp[:], st_ev[:, NT:2 * NT])
        nc.vector.tensor_sub(tmp2[:], st_od[:, 0:NT], Mx[:])
        nc.scalar.activation(tmp2[:], tmp2[:], AF.Exp)
        nc.vector.tensor_mul(tmp2[:], tmp2[:], st_od[:, NT:2 * NT])
        nc.vector.tensor_add(D[:], tmp[:], tmp2[:])
        nc.scalar.activation(tmp2[:], Mx[:], AF.Exp, scale=-1.0)     # e^-M
        nc.vector.tensor_mul(tmp[:], tmp2[:], cnt_s[:])
        nc.vector.tensor_add(D[:], D[:], tmp[:])
        nc.vector.reciprocal(rD[:], D[:])
        nc.vector.tensor_sub(tmp[:], mc[:], Mx[:])
        nc.scalar.activation(tmp[:], tmp[:], AF.Exp)                 # e^(m_own-M)
        nc.vector.tensor_mul(alpha[:], tmp[:], rD[:])
        nc.vector.tensor_mul(tmp2[:], tmp2[:], rD[:])
        nc.vector.tensor_copy(beta[:], tmp2[:])                      # bf16 cast

        va_T = []
        for t in range(NT):
            sb = ep.tile([128, VD], BF, tag=f"vaT{t}")
            nc.vector.tensor_scalar(sb[:], v_T[_cmap(t)][:], alpha[:, t:t + 1],
                                    None, OP.mult)
            va_T.append(sb)

        with tc.tile_pool(name="cspp", bufs=1, space="PSUM") as cs_pp:
            cs_ps = cs_pp.tile([VD, NT], F32, tag="cs")
            for t in range(NT):
                nc.tensor.matmul(cs_ps[:, t:t + 1], v_T[_cmap(t)][:],
                                 beta[:, t:t + 1], start=True, stop=True)
            cs_sb = sp.tile([VD, NT], F32, tag="cs_sb")
            nc.vector.tensor_copy(cs_sb[:], cs_ps[:])
        zz = sp.tile([VD, NT], F32, tag="zz")
        nc.vector.memset(zz[:], 0.0)
        pre = sp.tile([VD, NT], F32, tag="pre")
        nc.vector.tensor_tensor_scan(pre[:], cs_sb[:], zz[:], 0.0, OP.add, OP.add)
        suf = sp.tile([VD, NT], F32, tag="suf")
        nc.vector.tensor_scalar(suf[:], pre[:], pre[:, NT - 1:NT], -1.0,
                                OP.subtract, OP.mult)

        # ---- wv = V~ @ E (+ suffix for fully-masked region) ---------------
        with tc.tile_pool(name="wvpp", bufs=2, space="PSUM") as wv_pp, \
             tc.tile_pool(name="skpp", bufs=2, space="PSUM") as sk_pp, \
             tc.tile_pool(name="gw", bufs=2) as gw:
            for j in range(NCH):
                wv = wv_pp.tile([VD, CH], F32, tag="wv")
                tmax = min(NT - 1, 8 * j + 7)
                for t in range(tmax + 1):
                    ks = t // 2
                    lo = max(CH * j, 128 * ks)
                    eo = lo - 128 * ks
                    po = lo - CH * j
                    nc.tensor.matmul(wv[:, po:CH], va_T[t][:],
                                     e_tiles[t][:, eo:eo + (CH - po)],
                                     start=(t == 0), stop=(t == tmax))
                for kk in range(4 * j, 4 * j + 4):
                    if kk == NB - 1:
                        continue
                    o = 128 * (kk - 4 * j)
                    nc.vector.tensor_scalar(wv[:, o:o + 128], wv[:, o:o + 128],
                                            suf[:, 2 * kk + 1:2 * kk + 2],
                                            None, OP.add)
                # skip path for this chunk
                wv_sb = gw.tile([VD, CH], BF, tag="wv_sb")
                nc.vector.tensor_scalar(wv_sb[:], wv[:], 0.0, None, OP.add)
                e1w, e2w = concat_elu(gw, [wv_sb], "ew", fd=CH)
                sk = nin_mm(sk_pp, wsb['w_sk'], [e1w[0], e2w[0]],
                            _ptiles(NF), "skip", fd=CH)
                for i, ps in enumerate(sk):
                    sl = slice(CH * j, CH * (j + 1))
                    nc.vector.tensor_add(x1o_t[i][:, sl], ps[:], x1o_t[i][:, sl])

        ep_ctx.__exit__(None, None, None)

        # ---- grn_out tail --------------------------------------------------
        with tc.tile_pool(name="go", bufs=2) as go, \
             tc.tile_pool(name="popp", bufs=2, space="PSUM") as po_pp:
            e1o, e2o = concat_elu(go, x1o_t, "mo")
            ekt = [e1o[0], e1o[1], e2o[0], e2o[1]]
            bb = nin_mm(po_pp, wsb['w_co_o'], ekt,
                        [(NF, 128), (NF + 128, 32)], "coo")
            sigs = []
            for i, (o, s) in enumerate(_ptiles(NF)):
                sig = go.tile([s, P], F32, tag=f"go_sig{i}")
                nc.scalar.activation(sig[:], bb[i][:], AF.Tanh, scale=0.5,
                                     bias=_bias_slice(bsb['b_co_o_bb'], o, s))
                nc.vector.tensor_scalar(sig[:], sig[:], 0.5, 0.5, OP.mult, OP.add)
                sigs.append(sig)
            aa = nin_mm(po_pp, wsb['w_co_o'], ekt,
                        [(0, 128), (128, 32)], "coo")
            for i, (o, s) in enumerate(_ptiles(NF)):
                gg = go.tile([s, P], BF, tag="go_g")
                of = go.tile([s, P], F32, tag=f"go_o{i}")
                nc.vector.scalar_tensor_tensor(
                    gg[:], aa[i][:], _bias_slice(bsb['b_co_o_aa'], o, s),
                    sigs[i][:], OP.add, OP.mult)
                nc.vector.tensor_add(of[:], gg[:], ulf_t[i][:])
                nc.sync.dma_start(d['out'][o:o + s, :], of[:])


# ----------------------------------------------------------------------------
# entry point
# ----------------------------------------------------------------------------

_NC_CACHE = {}


def get_nc():
    if 'nc' not in _NC_CACHE:
        _NC_CACHE['nc'] = build()
    return _NC_CACHE['nc']


def kernel(x, ul, b, params):
    in_maps = host_prepare(x, ul, b, params)
    nc = get_nc()
    res = run_bass_kernel_spmd(nc, in_maps, core_ids=list(range(8)))
    return assemble( [r['out'] for r in res.results] )


def assemble(shards):
    full = np.zeros((N, NF, HW), np.float32)
    for c in range(8):
        s, p = c // 2, c % 2
        sh = np.asarray(shards[c])
        for k in range(NB):
            gblk = 2 * k + p
            full[s][:, 128 * gblk:128 * (gblk + 1)] = sh[:, 128 * k:128 * (k + 1)]
    return full.reshape(N, NF, 64, 64)


# revision 23
# speedup vs baseline: 1.3529x; 1.3529x over previous
"""Trainium2 Bass kernel for nn_AttentionBlock (PixelSNAIL-style attention).

Sharding: 8 cores; pair (2s, 2s+1) handles sample s. Within a pair, positions
(hw=4096) are split into 32 blocks of 128; core parity p owns blocks p::2
(interleaved for load balance, packed locally into 2048 columns).

Each core computes K/Q/V + grn for its 2048 positions; Q/V are pair-AllGathered;
attention scores S = Q^T K for all 4096 q-rows x 2048 local columns, with a
flash-style softmax: per-row local max (clamped at 0, matching the reference's
multiplicative-mask semantics), local exp-sums, a small stats AllGather (split
in two halves, pipelined against the S loop and the first V@W pass) and an
analytic combine.  The fully-masked region (exp(0)=1 entries) is folded in via
suffix sums of beta-scaled V column sums.  Output positions are local, so no
large cross-core reduction is needed.

All 8 cores run ONE program; parity differences live in input data only.
All weights/biases/constants are packed into two blob tensors -> 2 big DMAs.
"""
import numpy as np
import ml_dtypes

import concourse.bass as bass
import concourse.bacc as bacc
import concourse.mybir as mybir
import concourse.tile as tile
from concourse.bass_utils import run_bass_kernel_spmd

BF = mybir.dt.float16
F32 = mybir.dt.float32
NPBF = np.float16
AF = mybir.ActivationFunctionType
OP = mybir.AluOpType
AX = mybir.AxisListType

N, NF, KD, VD = 4, 160, 16, 80
HW = 4096
P = 2048
NB = 16
NT = 32
NTH = 16              # stats half size
CK, CQ = 169, 166
CH = 512
NCH = 4
PAIRS = [[0, 1], [2, 3], [4, 5], [6, 7]]

WKT = {
    'w_ci_k': [(0, 128), (128, 41), (CK, 128), (CK + 128, 41)],
    'w_ci_v': [(0, 128), (128, 41), (CK, 128), (CK + 128, 41)],
    'w_ci_q': [(0, 128), (128, 41), (CK, 128), (CK + 128, 41)],
    'w_ci_o': [(0, 128), (128, 41), (CK, 128), (CK + 128, 41)],
    'w_co_k': [(0, 128), (128, 41), (CK, 128), (CK + 128, 41)],
    'w_co_v': [(0, 128), (128, 41), (CK, 128), (CK + 128, 41)],
    'w_co_q': [(0, 128), (128, 38), (CQ, 128), (CQ + 128, 38)],
    'w_co_o': [(0, 128), (128, 32), (NF, 128), (NF + 128, 32)],
    'w_sk': [(0, VD), (VD, VD)],
    'w_nk': [(0, 128), (128, 41)],
    'w_nq': [(0, 128), (128, 38)],
    'w_nv': [(0, 128), (128, 41)],
}
WSHAPE = {
    'w_ci_k': (2 * CK, CK), 'w_ci_v': (2 * CK, CK),
    'w_ci_q': (2 * CK, CQ), 'w_ci_o': (2 * CK, NF),
    'w_co_k': (2 * CK, 2 * CK), 'w_co_v': (2 * CK, 2 * CK),
    'w_co_q': (2 * CQ, 2 * CQ), 'w_co_o': (2 * NF, 2 * NF),
    'w_nk': (CK, KD), 'w_nq': (CQ, KD), 'w_nv': (CK, VD),
    'w_sk': (2 * VD, NF),
}
BSPEC = {
    'b_ci_k': CK, 'b_ci_v': CK, 'b_ci_q': CQ, 'b_ci_o': NF,
    'b_co_k_aa': CK, 'b_co_k_bb': CK, 'b_co_v_aa': CK, 'b_co_v_bb': CK,
    'b_co_q_aa': CQ, 'b_co_q_bb': CQ, 'b_co_o_aa': NF, 'b_co_o_bb': NF,
    'b_nk': KD, 'b_nq': KD, 'b_nv': VD,
}


def _ptiles(c):
    return [(o, min(128, c - o)) for o in range(0, c, 128)]


def _cmap(g):
    return g // 2 if g % 2 == 0 else NB + g // 2


def _blob_layout():
    woff, off = {}, 0
    for k, kt in WKT.items():
        dout = WSHAPE[k][1]
        for i, (o, s) in enumerate(kt):
            woff[(k, i)] = (off, s, dout)
            off += dout
    woff['dmask'] = (off, 128, 256)
    off += 256
    woff['maskfull'] = (off, 128, 2 * 2048)
    off += 2 * 2048
    wtot = off
    boff, off = {}, 0
    for k, c in BSPEC.items():
        for i, (o, s) in enumerate(_ptiles(c)):
            boff[(k, i)] = (off, s)
            off += 1
    boff['cnt'] = (off, NT)
    off += NT
    return woff, wtot, boff, off


WOFF, WTOT, BOFF, BTOT = _blob_layout()


# ----------------------------------------------------------------------------
# host-side prep
# ----------------------------------------------------------------------------

def _wn(p):
    v = np.asarray(p['v'], np.float64)
    g = np.asarray(p['g'], np.float64)
    b = np.asarray(p['b'], np.float64)
    w = v * (g / np.linalg.norm(v, axis=1))[:, None]
    return w, b


def _pad_rows_to_A(w, src_c, dst):
    out = np.zeros((2 * CK, w.shape[1]))
    out[dst, :] = w[:src_c]
    out[[CK + i for i in dst], :] = w[src_c:]
    return out


def host_prepare(x, ul, b, params):
    x = np.asarray(x, np.float32)
    ul = np.asarray(ul, np.float32)
    b = np.asarray(b, np.float32)

    W, B = {}, {}

    def nin_eff(p, fold):
        w, bb = _wn(p)
        if fold:
            bb = bb - w.sum(axis=0)
        return w, bb

    w, bb = nin_eff(params['grn_k']['conv_input'], True)
    W['w_ci_k'], B['b_ci_k'] = w, bb
    w, bb = nin_eff(params['grn_v']['conv_input'], True)
    W['w_ci_v'], B['b_ci_v'] = w, bb
    w, bb = nin_eff(params['grn_q']['conv_input'], True)
    W['w_ci_q'], B['b_ci_q'] = _pad_rows_to_A(w, CQ, list(range(3, 169))), bb
    w, bb = nin_eff(params['grn_out']['conv_input'], True)
    wsk, bsk = nin_eff(params['grn_out']['nin_skip'], True)
    W['w_ci_o'] = _pad_rows_to_A(w, NF, list(range(3, 163)))
    B['b_ci_o'] = bb + bsk
    W['w_sk'] = wsk

    for nm, key in (('k', 'grn_k'), ('v', 'grn_v'), ('q', 'grn_q'), ('o', 'grn_out')):
        w, bb = nin_eff(params[key]['conv_out'], True)
        nf2 = bb.shape[0] // 2
        W[f'w_co_{nm}'] = w
        B[f'b_co_{nm}_aa'], B[f'b_co_{nm}_bb'] = bb[:nf2], 0.5 * bb[nf2:]

    for nm in ('k', 'q', 'v'):
        w, bb = nin_eff(params[f'nin_{nm}'], False)
        W[f'w_n{nm}'], B[f'b_n{nm}'] = w, bb

    r = np.arange(128)
    strict_u = (r[None, :] > r[:, None]).astype(np.float32)

    wblob0 = np.zeros((128, WTOT), NPBF)
    for k in WKT:
        for i, (o, s) in enumerate(WKT[k]):
            c, s_, dout = WOFF[(k, i)]
            wblob0[0:s, c:c + dout] = W[k][o:o + s, :].astype(NPBF)
    bblob = np.zeros((128, BTOT), np.float32)
    for k, cdim in BSPEC.items():
        for i, (o, s) in enumerate(_ptiles(cdim)):
            c, _ = BOFF[(k, i)]
            bblob[0:s, c] = B[k][o:o + s].astype(np.float32)
    cnt = np.zeros((128, NT), np.float32)
    for t in range(NT):
        cnt[:, t] = 128 * t - (128 if t % 2 == 1 else 0)
    bblob[:, BOFF['cnt'][0]:BOFF['cnt'][0] + NT] = cnt

    xubf = np.concatenate([x.reshape(N, 3, HW), ul.reshape(N, NF, HW),
                           b.reshape(N, 6, HW)], axis=1)
    cores = []
    dmc = WOFF['dmask'][0]
    mfc = WOFF['maskfull'][0]
    for c in range(8):
        s, p = c // 2, c % 2
        cols = np.concatenate([np.arange(128 * (2 * k + p), 128 * (2 * k + p) + 128)
                               for k in range(NB)])
        xs = xubf[s][:, cols]
        wblob = wblob0.copy()
        if p == 0:
            dm = np.concatenate([strict_u, np.zeros((128, 128), np.float32)], axis=1)
        else:
            dm = np.concatenate([np.ones((128, 128), np.float32), strict_u], axis=1)
        wblob[:, dmc:dmc + 256] = dm.astype(NPBF)
        mf = np.ones((128, 2 * 2048), np.float32)
        mf[:, 0:128] = dm[:, 0:128]
        mf[:, 2048:2048 + 128] = dm[:, 128:256]
        wblob[:, mfc:mfc + 2 * 2048] = mf.astype(NPBF)
        m = {
            'xub': np.ascontiguousarray(xs.astype(NPBF)),
            'ulb': np.ascontiguousarray(xs[3:169].astype(NPBF)),
            'ulf': np.ascontiguousarray(xs[3:163], np.float32),
            'wblob': wblob,
            'bblob': bblob,
        }
        cores.append(m)
    return cores


# ----------------------------------------------------------------------------
# device kernel
# ----------------------------------------------------------------------------

def build(debug=False):
    nc = bacc.Bacc("TRN2", target_bir_lowering=False, debug=debug,
                   enable_asserts=False, num_devices=8)

    d = {}
    d['xub'] = nc.dram_tensor('xub', [CK, P], BF, kind="ExternalInput")
    d['ulb'] = nc.dram_tensor('ulb', [CQ, P], BF, kind="ExternalInput")
    d['ulf'] = nc.dram_tensor('ulf', [NF, P], F32, kind="ExternalInput")
    d['wblob'] = nc.dram_tensor('wblob', [128, WTOT], BF, kind="ExternalInput")
    d['bblob'] = nc.dram_tensor('bblob', [128, BTOT], F32, kind="ExternalInput")
    d['out'] = nc.dram_tensor('out', [NF, P], F32, kind="ExternalOutput")
    d['qb'] = nc.dram_tensor('qb', [KD, P], BF)
    d['vb'] = nc.dram_tensor('vb', [VD, P], BF)
    d['stb1'] = nc.dram_tensor('stb1', [128, 2 * NTH], F32)
    d['stb2'] = nc.dram_tensor('stb2', [128, 2 * NTH], F32)
    d['qag'] = nc.dram_tensor('qag', [2 * KD, P], BF)
    d['vag'] = nc.dram_tensor('vag', [2 * VD, P], BF)
    d['st1ag'] = nc.dram_tensor('st1ag', [256, 2 * NTH], F32)
    d['st2ag'] = nc.dram_tensor('st2ag', [256, 2 * NTH], F32)

    with tile.TileContext(nc) as tc:
        _body(nc, tc, d)
    nc.compile()
    return nc


def nin_mm(nc, pool, w_tiles, act_tiles, m_tiles, ps_tag, fd=P):
    outs = []
    for mo, ms in m_tiles:
        pt = pool.tile([ms, fd], F32, tag=ps_tag, name="mmps")
        for c in range(0, fd, CH):
            sl = slice(c, min(fd, c + CH))
            for ki in range(len(w_tiles)):
                nc.tensor.matmul(
                    pt[:, sl], w_tiles[ki][:, mo:mo + ms], act_tiles[ki][:, sl],
                    start=(ki == 0), stop=(ki == len(w_tiles) - 1))
        outs.append(pt)
    return outs


def _body(nc, tc, d):
    with (tc.tile_pool(name="wp", bufs=1) as wp,
          tc.tile_pool(name="sp", bufs=1) as sp):

        xub_t, ulb_t, ulf_t = [], [], []
        x1o_t = []
        k_loc = sp.tile([KD, P], BF, tag="k_loc")

        with tc.tile_pool(name="gpp", bufs=1) as gpp, \
             tc.tile_pool(name="gp", bufs=1) as gp, \
             tc.tile_pool(name="pp", bufs=2, space="PSUM") as pp:

            for i, (o, s) in enumerate(_ptiles(CK)):
                tl = gpp.tile([s, P], BF, tag=f"xub{i}", name=f"xub{i}")
                nc.sync.dma_start(tl[:], d['xub'][o:o + s, :])
                xub_t.append(tl)
            for i, (o, s) in enumerate(_ptiles(CQ)):
                tl = gpp.tile([s, P], BF, tag=f"ulb{i}", name=f"ulb{i}")
                nc.sync.dma_start(tl[:], d['ulb'][o:o + s, :])
                ulb_t.append(tl)

            wblob_t = wp.tile([128, WTOT], BF, tag="wblob")
            NSPLIT = 8
            wchunk = (WTOT + NSPLIT - 1) // NSPLIT
            for ci_ in range(NSPLIT):
                lo = ci_ * wchunk
                hi = min(WTOT, lo + wchunk)
                nc.sync.dma_start(wblob_t[:, lo:hi], d['wblob'][:, lo:hi])
            bblob_t = wp.tile([128, BTOT], F32, tag="bblob")
            nc.sync.dma_start(bblob_t[:], d['bblob'][:, :])

            for i, (o, s) in enumerate(_ptiles(NF)):
                tl = sp.tile([s, P], F32, tag=f"ulf{i}", name=f"ulf{i}")
                nc.sync.dma_start(tl[:], d['ulf'][o:o + s, :])
                ulf_t.append(tl)

            def wv_(k, i):
                c, s, dout = WOFF[(k, i)]
                return wblob_t[0:s, c:c + dout]

            def wtiles(k):
                return [wv_(k, i) for i in range(len(WKT[k]))]

            def bias(k, off, size):
                ti, ro = off // 128, off % 128
                c, s = BOFF[(k, ti)]
                assert ro + size <= s
                return bblob_t[ro:ro + size, c:c + 1]

            d['_wv'] = wv_
            d['_bias'] = bias
            dmc = WOFF['dmask'][0]
            dmask_s = wblob_t[:, dmc:dmc + 256]
            mfc = WOFF['maskfull'][0]
            maskfull_s = wblob_t[:, mfc:mfc + 2 * 2048]
            cntc = BOFF['cnt'][0]
            cnt_s = bblob_t[:, cntc:cntc + NT]

            def concat_elu(pool, src_tiles, bt, fd=P, opool=None):
                op = opool or pool
                e1, e2 = [], []
                for i, s in enumerate(src_tiles):
                    sh = [s.shape[0], fd]
                    mx = pool.tile(sh, BF, tag=f"{bt}_mx", bufs=2, name="ce_mx")
                    a = pool.tile(sh, BF, tag=f"{bt}_a", bufs=2, name="ce_a")
                    w = pool.tile(sh, BF, tag=f"{bt}_w", bufs=2, name="ce_w")
                    o1 = op.tile(sh, BF, tag=f"{bt}_e1_{i}", name="ce_e1")
                    o2 = op.tile(sh, BF, tag=f"{bt}_e2_{i}", name="ce_e2")
                    nc.vector.tensor_scalar(mx[:], s[:], -1.0, None, OP.mult)
                    nc.vector.tensor_tensor(a[:], s[:], mx[:], OP.min)
                    nc.scalar.activation(w[:], a[:], AF.Exp)
                    nc.vector.scalar_tensor_tensor(o1[:], s[:], 1.0, w[:],
                                                   OP.add, OP.max)
                    nc.vector.scalar_tensor_tensor(o2[:], mx[:], 1.0, w[:],
                                                   OP.add, OP.max)
                    e1.append(o1)
                    e2.append(o2)
                return e1, e2

            d['_celu'] = concat_elu

            # ================= GRN stage ====================================
            e1A, e2A = concat_elu(gpp, xub_t, "A", opool=sp)
            A_kt = [e1A[0], e1A[1], e2A[0], e2A[1]]

            def branch(nm, orig_tiles, nf):
                ci = nin_mm(nc, pp, wtiles(f'w_ci_{nm}'), A_kt, _ptiles(nf), "mm")
                xs = []
                for i, ps in enumerate(ci):
                    xsb = gp.tile(list(ps.shape), BF, tag="mx_t", bufs=2, name="xsb")
                    nc.scalar.activation(xsb[:], ps[:], AF.Identity,
                                         bias=bias(f'b_ci_{nm}', i * 128,
                                                   ps.shape[0]))
                    xs.append(xsb)
                e1, e2 = concat_elu(gp, xs, "me")
                ekt = [e1[0], e1[1], e2[0], e2[1]]
                wco = wtiles(f'w_co_{nm}')
                bb = nin_mm(nc, pp, wco, ekt,
                            [(nf, 128), (nf + 128, nf - 128)], "mm")
                sigs = []
                for i in range(2):
                    ms = orig_tiles[i].shape[0]
                    sig = gp.tile([ms, P], F32, tag=f"sig{i}", name="sig")
                    nc.scalar.activation(sig[:], bb[i][:], AF.Tanh, scale=0.5,
                                         bias=bias(f'b_co_{nm}_bb', i * 128, ms))
                    nc.vector.tensor_scalar(sig[:], sig[:], 0.5, 0.5,
                                            OP.mult, OP.add)
                    sigs.append(sig)
                aa = nin_mm(nc, pp, wco, ekt, [(0, 128), (128, nf - 128)], "mm")
                outs = []
                for i in range(2):
                    ms = orig_tiles[i].shape[0]
                    gg = gp.tile([ms, P], BF, tag="gg", bufs=2, name="gg")
                    xo = gp.tile([ms, P], BF, tag=f"xo{i}", name="xo")
                    nc.vector.scalar_tensor_tensor(
                        gg[:], aa[i][:], bias(f'b_co_{nm}_aa', i * 128, ms),
                        sigs[i][:], OP.add, OP.mult)
                    nc.vector.tensor_tensor(xo[:], gg[:], orig_tiles[i][:], OP.add)
                    outs.append(xo)
                return outs

            def proj(nm, act_tiles, dout, dst_sb):
                ps = nin_mm(nc, pp, wtiles(f'w_n{nm}'), act_tiles,
                            [(0, dout)], "mm")[0]
                nc.vector.tensor_scalar(dst_sb[:], ps[:], bias(f'b_n{nm}', 0, dout),
                                        None, OP.add)

            xq = branch('q', ulb_t, CQ)
            q_loc = gpp.tile([KD, P], BF, tag="q_loc")
            proj('q', xq, KD, q_loc)
            nc.sync.dma_start(d['qb'][:, :], q_loc[:])
            nc.gpsimd.collective_compute(
                "AllGather", OP.bypass, replica_groups=PAIRS,
                ins=[d['qb'].ap().opt()], outs=[d['qag'].ap().opt()])

            xv = branch('v', xub_t, CK)
            v_loc = gpp.tile([VD, P], BF, tag="v_loc")
            proj('v', xv, VD, v_loc)
            nc.sync.dma_start(d['vb'][:, :], v_loc[:])
            nc.gpsimd.collective_compute(
                "AllGather", OP.bypass, replica_groups=PAIRS,
                ins=[d['vb'].ap().opt()], outs=[d['vag'].ap().opt()])

            xk = branch('k', xub_t, CK)
            proj('k', xk, KD, k_loc)

            A_sp = A_kt

        wv_ = d['_wv']
        bias = d['_bias']
        concat_elu = d['_celu']

        # ================= attention ========================================
        ep_ctx = tc.tile_pool(name="ep", bufs=1)
        ep = ep_ctx.__enter__()

        v_T = []
        for c in range(NT):
            vt = ep.tile([128, VD], BF, tag=f"vT{c}", name=f"vT{c}")
            if c < NB:
                src = d['vag'][0:VD, 128 * c:128 * (c + 1)]
            else:
                src = d['vag'][VD:2 * VD, 128 * (c - NB):128 * (c - NB + 1)]
            nc.sync.dma_start_transpose(vt[:], src)
            v_T.append(vt)

        negm1 = sp.tile([128, NTH], F32, tag="negm1")
        negm2 = sp.tile([128, NTH], F32, tag="negm2")
        mc1 = sp.tile([128, NTH], F32, tag="mc1")
        mc2 = sp.tile([128, NTH], F32, tag="mc2")
        ds1 = sp.tile([128, NTH], F32, tag="ds1")
        ds2 = sp.tile([128, NTH], F32, tag="ds2")
        e_tiles = []

        with tc.tile_pool(name="gc", bufs=1) as gc:
            q_all = gc.tile([KD, HW], BF, tag="q_all")
            nc.sync.dma_start(q_all[:, 0:P], d['qag'][0:KD, :])
            nc.sync.dma_start(q_all[:, P:HW], d['qag'][KD:2 * KD, :])

            with tc.tile_pool(name="spp", bufs=2, space="PSUM") as s_pp:
                for t in range(NT):
                    ks = t // 2
                    w = P - 128 * ks
                    negm = negm1 if t < NTH else negm2
                    dsum = ds1 if t < NTH else ds2
                    th = t % NTH
                    sps = s_pp.tile([128, P], F32, tag="s_ps", name="s_ps")
                    for c in range(0, w, CH):
                        c1 = min(w, c + CH)
                        nc.tensor.matmul(
                            sps[:, c:c1],
                            q_all[:, 128 * _cmap(t):128 * _cmap(t) + 128],
                            k_loc[:, 128 * ks + c:128 * ks + c1],
                            start=True, stop=True)
                    moff = 2048 * (t % 2)
                    mcH = mc1 if t < NTH else mc2
                    nc.vector.tensor_tensor_reduce(
                        sps[:, 0:w], sps[:, 0:w], maskfull_s[:, moff:moff + w],
                        1.0, 0.0, OP.mult, OP.max, mcH[:, th:th + 1])
                    nc.vector.tensor_scalar(negm[:, th:th + 1], mcH[:, th:th + 1],
                                            -1.0, None, OP.mult)
                    et = ep.tile([128, w], BF, tag=f"E{t}", name=f"E{t}")
                    nc.scalar.activation(et[:], sps[:, 0:w], AF.Exp,
                                         bias=negm[:, th:th + 1],
                                         accum_out=dsum[:, th:th + 1])
                    e_tiles.append(et)

                    if t == NTH - 1:
                        nc.sync.dma_start(d['stb1'][:, 0:NTH], mc1[:])
                        nc.sync.dma_start(d['stb1'][:, NTH:2 * NTH], ds1[:])
                        nc.gpsimd.collective_compute(
                            "AllGather", OP.bypass, replica_groups=PAIRS,
                            ins=[d['stb1'].ap().opt()],
                            outs=[d['st1ag'].ap().opt()])
                nc.sync.dma_start(d['stb2'][:, 0:NTH], mc2[:])
                nc.sync.dma_start(d['stb2'][:, NTH:2 * NTH], ds2[:])
                nc.gpsimd.collective_compute(
                    "AllGather", OP.bypass, replica_groups=PAIRS,
                    ins=[d['stb2'].ap().opt()], outs=[d['st2ag'].ap().opt()])

        # conv_input for grn_out -- fills the stats-AG window
        with tc.tile_pool(name="pp2", bufs=2, space="PSUM") as pp2:
            cio = nin_mm(nc, pp2, [wv_('w_ci_o', i) for i in range(4)], A_sp,
                         _ptiles(NF), "cio")
            for i, ps in enumerate(cio):
                xsb = sp.tile(list(ps.shape), BF, tag=f"x1o{i}", name="x1o")
                nc.scalar.activation(xsb[:], ps[:], AF.Identity,
                                     bias=bias('b_ci_o', i * 128, ps.shape[0]))
                x1o_t.append(xsb)

        # ---- stats combine (per half) --------------------------------------
        def combine(stag_d, mcH, nt0, tag):
            st_ev = sp.tile([128, 2 * NTH], F32, tag=f"se{tag}", name="st_ev")
            st_od = sp.tile([128, 2 * NTH], F32, tag=f"so{tag}", name="st_od")
            nc.sync.dma_start(st_ev[:], stag_d[0:128, :])
            nc.sync.dma_start(st_od[:], stag_d[128:256, :])
            Mx = sp.tile([128, NTH], F32, tag=f"Mx{tag}", name="Mx")
            tmp = sp.tile([128, NTH], F32, tag=f"tmp{tag}", name="tmp")
            tmp2 = sp.tile([128, NTH], F32, tag=f"tmp2{tag}", name="tmp2")
            D = sp.tile([128, NTH], F32, tag=f"D{tag}", name="D")
            alpha = sp.tile([128, NTH], F32, tag=f"al{tag}", name="alpha")
            beta = sp.tile([128, NTH], BF, tag=f"be{tag}", name="beta")
            rD = sp.tile([128, NTH], F32, tag=f"rD{tag}", name="rD")
            nc.vector.tensor_max(Mx[:], st_ev[:, 0:NTH], st_od[:, 0:NTH])
            nc.vector.tensor_sub(tmp[:], st_ev[:, 0:NTH], Mx[:])
            nc.scalar.activation(tmp[:], tmp[:], AF.Exp)
            nc.vector.tensor_mul(tmp[:], tmp[:], st_ev[:, NTH:2 * NTH])
            nc.vector.tensor_sub(tmp2[:], st_od[:, 0:NTH], Mx[:])
            nc.scalar.activation(tmp2[:], tmp2[:], AF.Exp)
            nc.vector.tensor_mul(tmp2[:], tmp2[:], st_od[:, NTH:2 * NTH])
            nc.vector.tensor_add(D[:], tmp[:], tmp2[:])
            nc.scalar.activation(tmp2[:], Mx[:], AF.Exp, scale=-1.0)
            nc.vector.tensor_mul(tmp[:], tmp2[:], cnt_s[:, nt0:nt0 + NTH])
            nc.vector.tensor_add(D[:], D[:], tmp[:])
            nc.vector.reciprocal(rD[:], D[:])
            nc.vector.tensor_sub(tmp[:], mcH[:], Mx[:])
            nc.scalar.activation(tmp[:], tmp[:], AF.Exp)
            nc.vector.tensor_mul(alpha[:], tmp[:], rD[:])
            nc.vector.tensor_mul(tmp2[:], tmp2[:], rD[:])
            nc.vector.tensor_copy(beta[:], tmp2[:])
            return alpha, beta

        alpha1, beta1 = combine(d['st1ag'], mc1, 0, "1")
        va_T = []
        for t in range(NTH):
            sb = ep.tile([128, VD], BF, tag=f"vaT{t}", name=f"vaT{t}")
            nc.vector.tensor_scalar(sb[:], v_T[_cmap(t)][:], alpha1[:, t:t + 1],
                                    None, OP.mult)
            va_T.append(sb)

        # ---- wv pass 1 (tiles 0..15) ---------------------------------------
        wv_ctx = tc.tile_pool(name="wvpp", bufs=1, space="PSUM")
        wv_pp = wv_ctx.__enter__()
        wv_chunks, tmaxs = [], []
        for j in range(NCH):
            wvp = wv_pp.tile([VD, CH], F32, tag=f"wv{j}", name=f"wv{j}")
            tmax = min(NT - 1, 8 * j + 7)
            tmaxs.append(tmax)
            for t in range(0, min(NTH, tmax + 1)):
                ks = t // 2
                lo = max(CH * j, 128 * ks)
                eo = lo - 128 * ks
                po = lo - CH * j
                nc.tensor.matmul(wvp[:, po:CH], va_T[t][:],
                                 e_tiles[t][:, eo:eo + (CH - po)],
                                 start=(t == 0), stop=(t == tmax))
            wv_chunks.append(wvp)

        alpha2, beta2 = combine(d['st2ag'], mc2, NTH, "2")
        for t in range(NTH, NT):
            sb = ep.tile([128, VD], BF, tag=f"vaT{t}", name=f"vaT{t}")
            nc.vector.tensor_scalar(sb[:], v_T[_cmap(t)][:],
                                    alpha2[:, t - NTH:t - NTH + 1], None, OP.mult)
            va_T.append(sb)

        with tc.tile_pool(name="cspp", bufs=1, space="PSUM") as cs_pp:
            cs_ps = cs_pp.tile([VD, NT], F32, tag="cs", name="cs")
            for t in range(NT):
                bcol = (beta1[:, t:t + 1] if t < NTH
                        else beta2[:, t - NTH:t - NTH + 1])
                nc.tensor.matmul(cs_ps[:, t:t + 1], v_T[_cmap(t)][:], bcol,
                                 start=True, stop=True)
            cs_sb = sp.tile([VD, NT], F32, tag="cs_sb")
            nc.vector.tensor_copy(cs_sb[:], cs_ps[:])
        zz = sp.tile([VD, NT], F32, tag="zz")
        nc.vector.memset(zz[:], 0.0)
        pre = sp.tile([VD, NT], F32, tag="pre")
        nc.vector.tensor_tensor_scan(pre[:], cs_sb[:], zz[:], 0.0, OP.add, OP.add)
        suf = sp.tile([VD, NT], F32, tag="suf")
        nc.vector.tensor_scalar(suf[:], pre[:], pre[:, NT - 1:NT], -1.0,
                                OP.subtract, OP.mult)

        # ---- wv pass 2 + suffix + skip path --------------------------------
        with tc.tile_pool(name="skpp", bufs=2, space="PSUM") as sk_pp, \
             tc.tile_pool(name="gw", bufs=2) as gw:
            for j in range(NCH):
                wvp = wv_chunks[j]
                tmax = tmaxs[j]
                for t in range(NTH, tmax + 1):
                    ks = t // 2
                    lo = max(CH * j, 128 * ks)
                    eo = lo - 128 * ks
                    po = lo - CH * j
                    nc.tensor.matmul(wvp[:, po:CH], va_T[t][:],
                                     e_tiles[t][:, eo:eo + (CH - po)],
                                     start=False, stop=(t == tmax))
                for kk in range(4 * j, 4 * j + 4):
                    if kk == NB - 1:
                        continue
                    o = 128 * (kk - 4 * j)
                    nc.vector.tensor_scalar(wvp[:, o:o + 128], wvp[:, o:o + 128],
                                            suf[:, 2 * kk + 1:2 * kk + 2],
                                            None, OP.add)
                wv_sb = gw.tile([VD, CH], BF, tag="wv_sb", name="wv_sb")
                nc.scalar.activation(wv_sb[:], wvp[:], AF.Copy)
                e1w, e2w = concat_elu(gw, [wv_sb], "ew", fd=CH)
                sk = nin_mm(nc, sk_pp, [wv_('w_sk', 0), wv_('w_sk', 1)],
                            [e1w[0], e2w[0]], _ptiles(NF), "skip", fd=CH)
                for i, ps in enumerate(sk):
                    sl = slice(CH * j, CH * (j + 1))
                    nc.vector.tensor_add(x1o_t[i][:, sl], ps[:], x1o_t[i][:, sl])

        ep_ctx.__exit__(None, None, None)
        wv_ctx.__exit__(None, None, None)

        # ---- grn_out tail --------------------------------------------------
        with tc.tile_pool(name="go", bufs=2) as go, \
             tc.tile_pool(name="popp", bufs=2, space="PSUM") as po_pp:
            e1o, e2o = concat_elu(go, x1o_t, "mo")
            ekt = [e1o[0], e1o[1], e2o[0], e2o[1]]
            wco = [wv_('w_co_o', i) for i in range(4)]
            bb = nin_mm(nc, po_pp, wco, ekt, [(NF, 128), (NF + 128, 32)], "coo")
            sigs = []
            for i, (o, s) in enumerate(_ptiles(NF)):
                sig = go.tile([s, P], F32, tag=f"go_sig{i}", name="go_sig")
                nc.scalar.activation(sig[:], bb[i][:], AF.Tanh, scale=0.5,
                                     bias=bias('b_co_o_bb', o, s))
                nc.vector.tensor_scalar(sig[:], sig[:], 0.5, 0.5, OP.mult, OP.add)
                sigs.append(sig)
            aa = nin_mm(nc, po_pp, wco, ekt, [(0, 128), (128, 32)], "coo")
            for i, (o, s) in enumerate(_ptiles(NF)):
                gg = go.tile([s, P], BF, tag="go_g", name="go_g")
                of = go.tile([s, P], F32, tag=f"go_o{i}", name="go_o")
                nc.vector.scalar_tensor_tensor(
                    gg[:], aa[i][:], bias('b_co_o_aa', o, s),
                    sigs[i][:], OP.add, OP.mult)
                nc.vector.tensor_add(of[:], gg[:], ulf_t[i][:])
                nc.sync.dma_start(d['out'][o:o + s, :], of[:])


# ----------------------------------------------------------------------------
# entry point
# ----------------------------------------------------------------------------

_NC_CACHE = {}


def get_nc():
    if 'nc' not in _NC_CACHE:
        _NC_CACHE['nc'] = build()
    return _NC_CACHE['nc']


def kernel(x, ul, b, params):
    in_maps = host_prepare(x, ul, b, params)
    nc = get_nc()
    res = run_bass_kernel_spmd(nc, in_maps, core_ids=list(range(8)))
    return assemble([r['out'] for r in res.results])


def assemble(shards):
    full = np.zeros((N, NF, HW), np.float32)
    for c in range(8):
        s, p = c // 2, c % 2
        sh = np.asarray(shards[c])
        for k in range(NB):
            gblk = 2 * k + p
            full[s][:, 128 * gblk:128 * (gblk + 1)] = sh[:, 128 * k:128 * (k + 1)]
    return full.reshape(N, NF, 64, 64)
